# revision 8
# baseline (speedup 1.0000x reference)
"""DescriptorLoss kernel for Trainium2 (8 NeuronCores, SPMD data-parallel).

Math (d' = 5*d, so hinges sit at d'=1 (neg branch) and d'=5 (pos branch)):
    loss*5*N = sum_{m=0} relu(d'-1) + 250 * sum_{m=1} relu(5-d')

Per core: shard = (batch, 16-row i-slab) -> 1024 ij rows x 4096 kl cols,
split into 8 groups (128 rows) x 4 ktiles (1024 cols) = 32 chunks.

Key identity: with t1 = d' - 1 - 1024*m  and  u = |t1|:
  - m=0: u = |d'-1| <= ~510;  m=1: u = 1025-d' in [~515, ~1535] (ranges
    separated, since |d'| < 9 sigma ~ 510).
  - hinge1 = sum relu(t1) = 0.5*(sum t1 + sum u); "sum t1" is linear (rank-1
    a.b sums + mask popcounts) -> host f64 side computation.
  - hinge2 = sum relu(u-1020) = sum u - sum min(u, 1020).

RC chunks (ScalarE egress): PE injects -1024*m into PSUM (extra matmul per
512 cols, idn=-I stationary, mask 0/1024 fp8 rhs); ACT activation(Abs,
bias=-1) PSUM->SBUF fp16 + accum(sum u). A per-group DVE tensor_scalar
(min 1020) at 4x over the contiguous RC columns yields sum min(u,1020).

RB chunks (VectorE egress, no inject): scalar_tensor_tensor computes signed
t1 = (d'-1) - 1024*m from PSUM (mask as fp8 tensor operand); two 4x-mode
tensor_scalar reduce ops give hinge1 = sum max(t1,0) and -hinge2 =
sum min(t1+1020, 0) directly.

Host combines everything in f64; loss = total / (5*B*IJ^2).
"""

import numpy as np
import ml_dtypes

import concourse.bacc as bacc
import concourse.mybir as mybir
import concourse.tile as tile
from concourse.bass_utils import run_bass_kernel_spmd

B, D, H, W = 2, 128, 64, 64
N_CORES = 8
IJ = H * W               # 4096
ROWS = IJ // 4           # 1024 rows per core
G = ROWS // 128          # 8 row groups
KT = 4                   # col chunks per group
KTILE = IJ // KT         # 1024
N_CHUNKS = G * KT        # 32
OMEGA = 1024.0
TH = OMEGA - 4.0         # 1020
RB3_GROUPS = (1, 3, 5)   # groups whose k=3 chunk also egresses on DVE

_cached = {}


def _is_rb(g, k):
    if k == 0:
        return True
    return k == 3 and g in RB3_GROUPS


def _rb_index(g, k):
    # dense index of an RB chunk among all RB chunks
    idx = 0
    for gg in range(G):
        for kk in range(KT):
            if not _is_rb(gg, kk):
                continue
            if (gg, kk) == (g, k):
                return idx
            idx += 1
    raise KeyError((g, k))


N_RB = sum(_is_rb(g, k) for g in range(G) for k in range(KT))


def _build_program():
    nc = bacc.Bacc("TRN2")
    f32 = mybir.dt.float32
    bf16 = mybir.dt.bfloat16
    f16 = mybir.dt.float16
    f8 = mybir.dt.float8e5
    Alu = mybir.AluOpType
    Act = mybir.ActivationFunctionType

    a5 = nc.declare_dram_parameter("a5", [D, ROWS], bf16, isOutput=False)
    bm = nc.declare_dram_parameter("bm", [D, IJ], bf16, isOutput=False)
    m8 = nc.declare_dram_parameter("m8", [ROWS, IJ], f8, isOutput=False)
    idn = nc.declare_dram_parameter("idn", [D, D], bf16, isOutput=False)
    accs_out = nc.declare_dram_parameter(
        "accs", [128, 2 * N_CHUNKS + G + 16], f32, isOutput=True)

    with tile.TileContext(nc) as tc:
        with (
            tc.tile_pool(name="desc", bufs=1) as desc_pool,
            tc.tile_pool(name="mask", bufs=8) as mask_pool,
            tc.tile_pool(name="t1", bufs=3) as t1_pool,
            tc.tile_pool(name="rb", bufs=2) as rb_pool,
            tc.tile_pool(name="junk", bufs=2) as junk_pool,
            tc.tile_pool(name="acc", bufs=1) as acc_pool,
            tc.tile_pool(name="ps", bufs=4, space="PSUM") as ps_pool,
        ):
            a_t = desc_pool.tile([D, ROWS], bf16, tag="a")
            b_t = desc_pool.tile([D, IJ], bf16, tag="b")
            id_t = desc_pool.tile([D, D], bf16, tag="idn")
            bias_t = desc_pool.tile([128, 1], f32, tag="bias")
            prime_t = desc_pool.tile([128, 1], f16, tag="prime")
            accA = acc_pool.tile([128, N_CHUNKS], f32, tag="accA")
            accB = acc_pool.tile([128, N_CHUNKS], f32, tag="accB")
            accC = acc_pool.tile([128, G], f32, tag="accC")
            accQ = acc_pool.tile([128, 16], f32, tag="accQ")

            nc.gpsimd.memset(bias_t[:], -1.0)
            nc.sync.dma_start(a_t[:, :128], a5[:, :128])
            nc.sync.dma_start(b_t[:, :KTILE], bm[:, :KTILE])
            nc.sync.dma_start(id_t[:], idn[:])
            # Prime the ACT table set (Abs) so the ~2.7us load overlaps DMAs.
            nc.scalar.activation(prime_t[:], bias_t[:], Act.Abs,
                                 bias=bias_t[:], scale=1.0)

            m_tiles = {}

            def load_mask(g, k):
                mt = mask_pool.tile([128, KTILE], f8, tag="m8")
                rs = slice(g * 128, (g + 1) * 128)
                ks = slice(k * KTILE, (k + 1) * KTILE)
                nc.sync.dma_start(mt[:], m8[rs, ks])
                m_tiles[(g, k)] = mt

            for k in range(KT):
                load_mask(0, k)

            t1_prev = None  # (g, tile, n_rc_cols) pending group min-op

            def group_min(entry):
                pg, pt, ncols = entry
                jk = junk_pool.tile([128, IJ], f16, tag="junk")
                nc.vector.tensor_scalar(
                    jk[:, :ncols], pt[:, KTILE:KTILE + ncols], TH, 0.0,
                    op0=Alu.min, op1=Alu.add,
                    accum_out=accC[:, pg:pg + 1],
                )

            for g in range(G):
                rs = slice(g * 128, (g + 1) * 128)
                t1_t = t1_pool.tile([128, IJ], f16, tag="t1")
                n_rc_cols = sum(
                    KTILE for k in range(KT) if not _is_rb(g, k))

                ps_tiles = {}
                for k in range(KT):
                    pst = ps_pool.tile([128, KTILE], f32, tag="d")
                    ps_tiles[k] = pst
                    if g == 0 and k == 1:
                        # rest of the descriptors: emitted before any matmul
                        # that consumes them, after k0 so PE starts early
                        nc.sync.dma_start(a_t[:, 128:], a5[:, 128:])
                        for kk in range(1, KT):
                            ksl = slice(kk * KTILE, (kk + 1) * KTILE)
                            nc.sync.dma_start(b_t[:, ksl], bm[:, ksl])
                    rb = _is_rb(g, k)
                    for h in range(KTILE // 512):
                        hs = slice(h * 512, (h + 1) * 512)
                        cs = slice(k * KTILE + h * 512,
                                   k * KTILE + (h + 1) * 512)
                        nc.tensor.matmul(pst[:, hs], a_t[:, rs], b_t[:, cs],
                                         start=True, stop=rb)

                if g + 1 < G:
                    for k in range(KT):
                        load_mask(g + 1, k)

                # PE mask injection for RC chunks (idn stationary, one LDW)
                for k in range(KT):
                    if _is_rb(g, k):
                        continue
                    mt = m_tiles[(g, k)]
                    pst = ps_tiles[k]
                    for h in range(KTILE // 512):
                        hs = slice(h * 512, (h + 1) * 512)
                        nc.tensor.matmul(pst[:, hs], id_t[:], mt[:, hs],
                                         start=False, stop=True)

                # egress: RB first (no inject dependency), then RC
                order = sorted(range(KT),
                               key=lambda k: 0 if _is_rb(g, k) else 1)
                for k in order:
                    cid = g * KT + k
                    pst = ps_tiles[k]
                    ksl = slice(k * KTILE, (k + 1) * KTILE)
                    if _is_rb(g, k):
                        ridx = _rb_index(g, k)
                        mt = m_tiles[(g, k)]
                        rb_t = rb_pool.tile([128, KTILE], f16, tag="rbt1")
                        rb_j = rb_pool.tile([128, KTILE], f16, tag="rbjunk")
                        nc.vector.scalar_tensor_tensor(
                            rb_t[:], pst[:], -1.0, mt[:],
                            op0=Alu.add, op1=Alu.subtract,
                        )
                        nc.vector.tensor_scalar(
                            rb_j[:], rb_t[:], 0.0, 0.0,
                            op0=Alu.max, op1=Alu.add,
                            accum_out=accB[:, cid:cid + 1],
                        )
                        nc.vector.tensor_scalar(
                            rb_j[:], rb_t[:], -TH, 0.0,
                            op0=Alu.min, op1=Alu.add,
                            accum_out=accQ[:, ridx:ridx + 1],
                        )
                    else:
                        nc.scalar.activation(
                            t1_t[:, ksl], pst[:], Act.Abs,
                            bias=bias_t[:], scale=1.0,
                            accum_out=accA[:, cid:cid + 1],
                        )

                # group min-op for the previous group (keeps DVE busy while
                # this group's ACT egresses run)
                if t1_prev is not None:
                    group_min(t1_prev)
                t1_prev = (g, t1_t, n_rc_cols)

            group_min(t1_prev)

            nc.sync.dma_start(accs_out[:, :N_CHUNKS], accA[:])
            nc.sync.dma_start(accs_out[:, N_CHUNKS:2 * N_CHUNKS], accB[:])
            nc.sync.dma_start(
                accs_out[:, 2 * N_CHUNKS:2 * N_CHUNKS + G], accC[:])
            nc.sync.dma_start(accs_out[:, 2 * N_CHUNKS + G:], accQ[:])

    nc.finalize()
    return nc


def _prep_inputs(descriptors_0, descriptors_1, similarity_mask):
    d0 = np.asarray(descriptors_0, dtype=np.float32)
    d1 = np.asarray(descriptors_1, dtype=np.float32)
    mkv = np.asarray(similarity_mask)
    idn = (-np.eye(D, dtype=np.float32)).astype(ml_dtypes.bfloat16)
    in_maps = []
    side = []
    for c in range(N_CORES):
        b = c >> 2
        isl = (c & 3) * 16
        a5 = (d0[b].reshape(D, IJ)[:, isl * W:(isl + 16) * W]
              * np.float32(5.0)).astype(ml_dtypes.bfloat16)
        bmv = d1[b].reshape(D, IJ).astype(ml_dtypes.bfloat16)
        mblk = mkv[b, isl:isl + 16].reshape(ROWS, IJ)
        m8v = (mblk.astype(np.float32) * np.float32(OMEGA)).astype(
            ml_dtypes.float8_e5m2)
        in_maps.append(
            {
                "a5": np.ascontiguousarray(a5),
                "bm": np.ascontiguousarray(bmv),
                "m8": np.ascontiguousarray(m8v),
                "idn": np.ascontiguousarray(idn),
            }
        )
        # linear term sum(t1) per RC chunk, f64 from the same bf16 values
        a64 = a5.astype(np.float64)
        b64 = bmv.astype(np.float64)
        s_lin_rc = 0.0
        for g in range(G):
            asum = a64[:, g * 128:(g + 1) * 128].sum(axis=1)
            for k in range(KT):
                if _is_rb(g, k):
                    continue
                bsum = b64[:, k * KTILE:(k + 1) * KTILE].sum(axis=1)
                n1 = float(mblk[g * 128:(g + 1) * 128,
                                k * KTILE:(k + 1) * KTILE].sum(dtype=np.int64))
                s_lin_rc += float(asum @ bsum) - OMEGA * n1 - 128.0 * KTILE
        side.append(s_lin_rc)
    _cached["side"] = side
    return in_maps


def _run(in_maps, **kwargs):
    if "nc" not in _cached:
        _cached["nc"] = _build_program()
    return run_bass_kernel_spmd(_cached["nc"], in_maps, list(range(N_CORES)),
                                **kwargs)


def _combine(results):
    side = _cached["side"]
    total = 0.0
    for r, s_lin_rc in zip(results, side):
        acc = r["accs"].astype(np.float64)
        accA = acc[:, :N_CHUNKS]
        accB = acc[:, N_CHUNKS:2 * N_CHUNKS]
        accC = acc[:, 2 * N_CHUNKS:2 * N_CHUNKS + G]
        accQ = acc[:, 2 * N_CHUNKS + G:]
        # RC chunks: hinge1 via linear+abs identity, hinge2 via min-op
        a_rc_tot = 0.0
        hinge2 = 0.0
        for g in range(G):
            a_g = sum(accA[:, g * KT + k].sum()
                      for k in range(KT) if not _is_rb(g, k))
            a_rc_tot += a_g
            hinge2 += a_g - accC[:, g].sum()
        hinge1 = 0.5 * (s_lin_rc + a_rc_tot)
        # RB chunks: direct hinge sums; accQ holds sum min(t1,-1020)
        for g in range(G):
            for k in range(KT):
                if not _is_rb(g, k):
                    continue
                hinge1 += accB[:, g * KT + k].sum()
                q = accQ[:, _rb_index(g, k)].sum() + TH * 128.0 * KTILE
                hinge2 -= q
        total += hinge1 + 250.0 * hinge2
    return np.float32(total / (5.0 * B * IJ * IJ))


def kernel(descriptors_0, descriptors_1, similarity_mask):
    in_maps = _prep_inputs(descriptors_0, descriptors_1, similarity_mask)
    res = _run(in_maps)
    return _combine(res.results)


# revision 9
# speedup vs baseline: 1.3330x; 1.3330x over previous
"""DescriptorLoss kernel for Trainium2 (8 NeuronCores, SPMD data-parallel).

Math (d' = 5*d, hinges at d'=1 (neg branch, m=0) and d'=5 (pos branch, m=1)):
    loss*5*N = sum_{m=0} relu(d'-1) + 250 * sum_{m=1} relu(5-d')

Per core: shard = (batch, 16-row i-slab) -> 1024 ij rows x 4096 kl cols,
8 groups (128 rows) x 2 pairs (2048 cols) = 16 pair-tiles.

Identity: with dM = d' - 1024*m (PE-injected mask offset), t1 = dM - 1,
u = |t1|:
  - m=0: u = |d'-1| <= ~510;  m=1: u = 1025-d' in [~515, ~1535]
    (ranges separated; |d'| < 9 sigma ~ 510).
  - hinge1 = sum relu(t1) = 0.5*(sum t1 + sum u); sum t1 is linear
    (rank-1 a.b sums + mask popcount) -> host f64.
  - hinge2 = sum relu(u-1020) = sum u - sum min(u, 1020).

Pipeline per pair [128 x 2048]:
  PE:  4 mains (a5 stationary) + 4 injects (idn=-I stationary, mask 0/1024
       fp8 moving) -> PSUM fp32 dM.
  ACT: activation(Abs, bias=-1) PSUM->SBUF fp16 u-tile + accum_out = sum u.
  DVE: per group, one tensor_scalar (min 1020, add-reduce) FD=4096 over the
       u-tile -> sum min(u, 1020).
Host combines in f64; loss = total / (5*B*IJ^2).
"""

import numpy as np
import ml_dtypes

import concourse.bacc as bacc
import concourse.mybir as mybir
import concourse.tile as tile
from concourse.bass_utils import run_bass_kernel_spmd

B, D, H, W = 2, 128, 64, 64
N_CORES = 8
IJ = H * W               # 4096
ROWS = IJ // 4           # 1024 rows per core
G = ROWS // 128          # 8 row groups
PAIR = 2048              # egress tile width
N_PAIRS = G * 2          # 16
OMEGA = 1024.0
TH = OMEGA - 4.0         # 1020

_cached = {}


def _build_program():
    nc = bacc.Bacc("TRN2")
    f32 = mybir.dt.float32
    bf16 = mybir.dt.bfloat16
    f16 = mybir.dt.float16
    f8 = mybir.dt.float8e5
    Alu = mybir.AluOpType
    Act = mybir.ActivationFunctionType

    a5 = nc.declare_dram_parameter("a5", [D, ROWS], bf16, isOutput=False)
    bm = nc.declare_dram_parameter("bm", [D, IJ], bf16, isOutput=False)
    m8 = nc.declare_dram_parameter("m8", [ROWS, IJ], f8, isOutput=False)
    idn = nc.declare_dram_parameter("idn", [D, D], bf16, isOutput=False)
    accs_out = nc.declare_dram_parameter(
        "accs", [128, N_PAIRS + G], f32, isOutput=True)

    with tile.TileContext(nc) as tc:
        with (
            tc.tile_pool(name="desc", bufs=1) as desc_pool,
            tc.tile_pool(name="mask", bufs=4) as mask_pool,
            tc.tile_pool(name="t1", bufs=3) as t1_pool,
            tc.tile_pool(name="junk", bufs=2) as junk_pool,
            tc.tile_pool(name="acc", bufs=1) as acc_pool,
            tc.tile_pool(name="ps", bufs=2, space="PSUM") as ps_pool,
        ):
            a_t = desc_pool.tile([D, ROWS], bf16, tag="a")
            b_t = desc_pool.tile([D, IJ], bf16, tag="b")
            id_t = desc_pool.tile([D, D], bf16, tag="idn")
            bias_t = desc_pool.tile([128, 1], f32, tag="bias")
            prime_t = desc_pool.tile([128, 1], f16, tag="prime")
            accA = acc_pool.tile([128, N_PAIRS], f32, tag="accA")
            accC = acc_pool.tile([128, G], f32, tag="accC")

            nc.gpsimd.memset(bias_t[:], -1.0)
            nc.sync.dma_start(a_t[:, :128], a5[:, :128])
            nc.sync.dma_start(b_t[:, :PAIR], bm[:, :PAIR])
            nc.sync.dma_start(id_t[:], idn[:])
            # Prime the ACT table set (Abs): ~2.7us load overlaps early DMAs.
            nc.scalar.activation(prime_t[:], bias_t[:], Act.Abs,
                                 bias=bias_t[:], scale=1.0)

            m_tiles = {}

            def load_mask(g, p):
                mt = mask_pool.tile([128, PAIR], f8, tag="m8")
                rs = slice(g * 128, (g + 1) * 128)
                ks = slice(p * PAIR, (p + 1) * PAIR)
                nc.sync.dma_start(mt[:], m8[rs, ks])
                m_tiles[(g, p)] = mt

            load_mask(0, 0)
            load_mask(0, 1)

            t1_prev = None  # (g, tile) pending the group min-op

            def group_min(entry):
                pg, pt = entry
                jk = junk_pool.tile([128, IJ], f16, tag="junk")
                nc.vector.tensor_scalar(
                    jk[:], pt[:], TH, 0.0,
                    op0=Alu.min, op1=Alu.add,
                    accum_out=accC[:, pg:pg + 1],
                )

            for g in range(G):
                rs = slice(g * 128, (g + 1) * 128)
                t1_t = t1_pool.tile([128, IJ], f16, tag="t1")

                ps_tiles = {}
                # mains: a-block stationary across the whole group
                for p in range(2):
                    pst = ps_pool.tile([128, PAIR], f32, tag="d")
                    ps_tiles[p] = pst
                    if g == 0 and p == 1:
                        # remaining descriptors, before anything consumes them
                        nc.sync.dma_start(a_t[:, 128:], a5[:, 128:])
                        nc.sync.dma_start(b_t[:, PAIR:], bm[:, PAIR:])
                    for h in range(PAIR // 512):
                        hs = slice(h * 512, (h + 1) * 512)
                        cs = slice(p * PAIR + h * 512, p * PAIR + (h + 1) * 512)
                        nc.tensor.matmul(pst[:, hs], a_t[:, rs], b_t[:, cs],
                                         start=True, stop=False)

                if g + 1 < G:
                    load_mask(g + 1, 0)
                    load_mask(g + 1, 1)

                # mask injection, idn stationary across the whole group
                for p in range(2):
                    mt = m_tiles[(g, p)]
                    pst = ps_tiles[p]
                    for h in range(PAIR // 512):
                        hs = slice(h * 512, (h + 1) * 512)
                        nc.tensor.matmul(pst[:, hs], id_t[:], mt[:, hs],
                                         start=False, stop=True)

                # ACT egress per pair: u = |dM - 1| + accum(sum u)
                for p in range(2):
                    pid = g * 2 + p
                    nc.scalar.activation(
                        t1_t[:, p * PAIR:(p + 1) * PAIR], ps_tiles[p][:],
                        Act.Abs, bias=bias_t[:], scale=1.0,
                        accum_out=accA[:, pid:pid + 1],
                    )

                # group min-op for the previous group
                if t1_prev is not None:
                    group_min(t1_prev)
                t1_prev = (g, t1_t)

            group_min(t1_prev)

            nc.sync.dma_start(accs_out[:, :N_PAIRS], accA[:])
            nc.sync.dma_start(accs_out[:, N_PAIRS:], accC[:])

    nc.finalize()
    return nc


def _prep_inputs(descriptors_0, descriptors_1, similarity_mask):
    d0 = np.asarray(descriptors_0, dtype=np.float32)
    d1 = np.asarray(descriptors_1, dtype=np.float32)
    mkv = np.asarray(similarity_mask)
    idn = (-np.eye(D, dtype=np.float32)).astype(ml_dtypes.bfloat16)
    in_maps = []
    side = []
    for c in range(N_CORES):
        b = c >> 2
        isl = (c & 3) * 16
        a5 = (d0[b].reshape(D, IJ)[:, isl * W:(isl + 16) * W]
              * np.float32(5.0)).astype(ml_dtypes.bfloat16)
        bmv = d1[b].reshape(D, IJ).astype(ml_dtypes.bfloat16)
        mblk = mkv[b, isl:isl + 16].reshape(ROWS, IJ)
        m8v = (mblk.astype(np.float32) * np.float32(OMEGA)).astype(
            ml_dtypes.float8_e5m2)
        in_maps.append(
            {
                "a5": np.ascontiguousarray(a5),
                "bm": np.ascontiguousarray(bmv),
                "m8": np.ascontiguousarray(m8v),
                "idn": np.ascontiguousarray(idn),
            }
        )
        # linear term sum(t1) over the whole shard, f64 from the same
        # bf16 values the PE consumes
        asum = a5.astype(np.float64).sum(axis=1)
        bsum = bmv.astype(np.float64).sum(axis=1)
        s_dp = float(asum @ bsum)
        n1 = float(mblk.sum(dtype=np.int64))
        s_lin = s_dp - OMEGA * n1 - float(ROWS * IJ)
        side.append(s_lin)
    _cached["side"] = side
    return in_maps


def _run(in_maps, **kwargs):
    if "nc" not in _cached:
        _cached["nc"] = _build_program()
    return run_bass_kernel_spmd(_cached["nc"], in_maps, list(range(N_CORES)),
                                **kwargs)


def _combine(results):
    side = _cached["side"]
    total = 0.0
    for r, s_lin in zip(results, side):
        acc = r["accs"].astype(np.float64)
        accA = acc[:, :N_PAIRS]
        accC = acc[:, N_PAIRS:]
        a_tot = accA.sum()
        hinge1 = 0.5 * (s_lin + a_tot)
        hinge2 = 0.0
        for g in range(G):
            a_g = accA[:, 2 * g:2 * g + 2].sum()
            hinge2 += a_g - accC[:, g].sum()
        total += hinge1 + 250.0 * hinge2
    return np.float32(total / (5.0 * B * IJ * IJ))


def kernel(descriptors_0, descriptors_1, similarity_mask):
    in_maps = _prep_inputs(descriptors_0, descriptors_1, similarity_mask)
    res = _run(in_maps)
    return _combine(res.results)
